# revision 33
# baseline (speedup 1.0000x reference)
"""ALiBi causal attention on 8 TRN2 NeuronCores — no-communication variant.

Sharding: batch (4) x query-half (2) = 8 cores, zero collectives.
Each core receives a HOST-WINDOWED x covering key positions
[Q0-128, Q0+1024) of its batch (front-padded with zeros on even cores).
It computes K/V for the 9-k-tile window, Q for its half (reusing the
same x window — queries are window cols 128:1152), banded causal
attention (ALiBi decay zeroes k < q-127 in bf16), and the out-projection.

Key structure:
- K0 and K8 share one 256-col score slot -> 2 exp ops/head, no G2.
- attn output written straight into SBUF (at tile) by the DVE
  reciprocal+mult normalize; denominator rides the PV matmul via ones
  columns in vtile.
- all big inputs host-packed kt-major so each DMA is one contiguous
  run per partition (128 descriptors, matters under io-level dynamic
  DMA generation), spread over the sync/act/pool queues so the first
  K chain starts ~3.5us in.
- out-proj chains for mi 0-3 split into (kt 0-5) -> out and (kt 6,7)
  -> out2 partial passes (host adds them) so the PE stays busy while
  the DVE normalizes the trailing heads.
- masks on Pool, V/K/Q/C evacs + exp on Act, normalize on DVE.
"""
import numpy as np


def _bf16_dtype():
    import ml_dtypes

    return np.dtype(ml_dtypes.bfloat16)


B, S, D = 4, 2048, 1024
H, HD = 16, 64
NCORES = 8
QH = S // 2          # 1024 queries per core
KW = QH + 128        # 1152 key-window positions per core (9 k-tiles)
NKT = KW // 128      # 9 local k-tiles

_CACHE = {}


def _build():
    import concourse.mybir as mybir
    import concourse.tile as tile
    from concourse import bacc
    from contextlib import ExitStack

    F32 = mybir.dt.float32
    BF16 = mybir.dt.bfloat16
    AF = mybir.ActivationFunctionType
    MULT = mybir.AluOpType.mult
    DIV = mybir.AluOpType.divide

    nc = bacc.Bacc("TRN2", target_bir_lowering=False, debug=False, num_devices=NCORES)

    # All big inputs are host-packed [128, kt-major] so every DMA is one
    # contiguous run per partition: 128 descriptors instead of 1024. The
    # compile uses io-level dynamic DMA generation, so descriptors are
    # rebuilt every call - descriptor count is per-call overhead.
    xpk = nc.dram_tensor("xpk", [128, 8 * KW], BF16, kind="ExternalInput").ap()
    wqp = nc.dram_tensor("wqp", [128, 8 * D], BF16, kind="ExternalInput").ap()
    wkp = nc.dram_tensor("wkp", [128, 8 * D], BF16, kind="ExternalInput").ap()
    wvp = nc.dram_tensor("wvp", [128, 8 * D], BF16, kind="ExternalInput").ap()
    wop = nc.dram_tensor("wop", [128, 8 * D], BF16, kind="ExternalInput").ap()
    m2g0 = nc.dram_tensor("m2g0", [128, 1024], BF16, kind="ExternalInput").ap()
    m2c = nc.dram_tensor("m2c", [128, 256], BF16, kind="ExternalInput").ap()
    bqk = nc.dram_tensor("bqk", [128, 16], F32, kind="ExternalInput").ap()
    bo = nc.dram_tensor("bo", [128, 8], F32, kind="ExternalInput").ap()
    out = nc.dram_tensor("out", [D, QH], F32, kind="ExternalOutput").ap()
    # partial out-proj contributions (kt 6,7) for mi 0..3; host adds them.
    out2 = nc.dram_tensor("out2", [D // 2, QH], F32, kind="ExternalOutput").ap()

    # x is packed as three kt-major chunks (cols 0:256, 256:640, 640:1152)
    # so each chunk loads contiguously; chunk boundaries are chosen so
    # K, Q and V matmul column ranges never straddle a chunk.
    XCH = [(0, 256), (256, 384), (640, 512)]
    xsrc = []
    off = 0
    for s0, wd in XCH:
        xsrc.append(
            xpk[:, off : off + 8 * wd].rearrange("p (kt f) -> p kt f", kt=8)
        )
        off += 8 * wd
    wq3 = wqp.rearrange("p (kt f) -> p kt f", kt=8)     # [128, 8, 1024]
    wv3 = wvp.rearrange("p (kt f) -> p kt f", kt=8)
    wo3 = wop.rearrange("p (kt f) -> p kt f", kt=8)
    # wk is packed as four mi-pair chunks of [8 kt x 256]
    wksrc = [
        wkp[:, 2048 * p : 2048 * p + 2048].rearrange("p2 (kt f) -> p2 kt f", kt=8)
        for p in range(4)
    ]

    with tile.TileContext(nc) as tc:
        with (
            tc.tile_pool(name="const", bufs=1) as cpool,
            tc.tile_pool(name="big", bufs=1) as big,
            ExitStack() as outer,
        ):
            m2g0_sb = cpool.tile([128, 1024], BF16)
            m2_sb = cpool.tile([128, 256], BF16)
            bqk_sb = cpool.tile([128, 16], F32)
            bo_sb = cpool.tile([128, 8], F32)

            # persistent tiles
            ktile = big.tile([128, NKT, 8, 128], BF16, tag="kt", name="ktile")
            vtile = big.tile([128, NKT, H, 2 * HD], BF16, tag="vt", name="vtile")
            qt = big.tile([128, 8, QH], BF16, tag="qt", name="qtile")
            at = big.tile([128, 8, QH], BF16, tag="at", name="atile")
            wo_sb = big.tile([128, 8, D], BF16, tag="wo", name="wo_sb")

            # (vtile ones memsets are emitted later, after the startup DMAs,
            # so they don't block the DVE/Pool queues at t=0)

            # LIFO pool staging: psS/pt (whole stage B) below, then
            # psAV/w/x (until V-proj done), then psPV, then psC/yt.
            stageB = outer.enter_context(ExitStack())
            psS = stageB.enter_context(
                tc.tile_pool(name="psS", bufs=2, space="PSUM")
            )
            ptpool = stageB.enter_context(tc.tile_pool(name="pt", bufs=4))
            denpool = stageB.enter_context(tc.tile_pool(name="den", bufs=3))
            psPV = stageB.enter_context(
                tc.tile_pool(name="psPV", bufs=1, space="PSUM")
            )
            stageWX = outer.enter_context(ExitStack())
            psAV = stageWX.enter_context(
                tc.tile_pool(name="psAV", bufs=2, space="PSUM")
            )
            wpool = stageWX.enter_context(tc.tile_pool(name="w", bufs=1))
            xpool = stageWX.enter_context(tc.tile_pool(name="xin", bufs=1))

            wq_sb = wpool.tile([128, 8, D], BF16, tag="wq", name="wq_sb")
            wv_sb = wpool.tile([128, 8, D], BF16, tag="wv", name="wv_sb")
            wk_t = [
                wpool.tile([128, 8, 256], BF16, tag=f"wk{p}", name=f"wk{p}")
                for p in range(4)
            ]
            xc = [
                xpool.tile([128, 8, wd], BF16, tag=f"xc{i}", name=f"xc{i}")
                for i, (s0, wd) in enumerate(XCH)
            ]

            def xsl(c, w):
                """x slice [128, 8, w] at global window col c."""
                for (s0, wd), t in zip(XCH, xc):
                    if s0 <= c and c + w <= s0 + wd:
                        return t[:, :, c - s0 : c - s0 + w]
                raise AssertionError((c, w))

            # input DMAs, spread over the three DMA-capable queues so the
            # first K chain (x cols 0:256 + wk pair 0) is ready ~3.5us in
            # and later chains' operands stream in just ahead of use:
            #   pool: wk pair chunks, wq, wo, then a vtile ones memset
            #   sync: x chunks, bqk, wv + consts
            #   act:  only the exp-table load, pulled to t=0 by a 1-elem
            #         warm activation, so it doesn't delay the K evacs
            warm = cpool.tile([1, 1], F32)
            nc.vector.memset(warm[:], 0.0)
            nc.scalar.activation(warm[:], warm[:], AF.Exp)
            for p in range(4):
                nc.gpsimd.dma_start(wk_t[p][:], wksrc[p])
            nc.sync.dma_start(xc[0][:], xsrc[0])
            nc.sync.dma_start(bqk_sb[:], bqk)
            nc.sync.dma_start(xc[1][:], xsrc[1])
            nc.sync.dma_start(xc[2][:], xsrc[2])
            nc.gpsimd.dma_start(wq_sb[:], wq3)
            nc.sync.dma_start(wv_sb[:], wv3)
            nc.sync.dma_start(m2g0_sb[:], m2g0)
            nc.sync.dma_start(m2_sb[:], m2c)
            nc.sync.dma_start(bo_sb[:], bo)
            nc.gpsimd.dma_start(wo_sb[:], wo3)
            # ones columns for the replicated-denominator PV trick
            nc.vector.memset(vtile[:, :, 0:8, HD : 2 * HD], 1.0)
            nc.gpsimd.memset(vtile[:, :, 8:16, HD : 2 * HD], 1.0)

            # ---- K-projection: key chunks (= x chunks) x 8 m-tiles. The
            # first chunk is small so the first chain starts as soon as
            # the lean startup DMA prefix lands. ----
            for ci, (s0, kw) in enumerate(XCH):
                nt = kw // 128
                for mi in range(8):
                    ps = psAV.tile([128, 512], F32, tag="a", name=f"kp{ci}_{mi}")
                    for kt in range(8):
                        nc.tensor.matmul(
                            ps[:, 0:kw],
                            wk_t[mi // 2][:, kt, (mi % 2) * 128 : (mi % 2) * 128 + 128],
                            xsl(s0, kw)[:, kt, :],
                            start=(kt == 0),
                            stop=(kt == 7),
                        )
                    nc.scalar.activation(
                        ktile[:, s0 // 128 : s0 // 128 + nt, mi, :],
                        ps[:, 0:kw].rearrange("p (t f) -> p t f", t=nt),
                        AF.Identity,
                        bias=bqk_sb[:, 8 + mi : 8 + mi + 1],
                    )
            # ---- Q-projection (deferred): queries are window cols
            # 128:1152, split per x chunk; Q(mi) is emitted just before
            # head 2*mi so the PE has surplus work against the Act-exp
            # critical path ----
            QCH = [(128, 128, 0), (256, 384, 128), (640, 512, 512)]

            def emit_q(mi):
                for qc, (s0, wd, d0) in enumerate(QCH):
                    ps = psAV.tile([128, 512], F32, tag="a", name=f"qp{qc}_{mi}")
                    for kt in range(8):
                        nc.tensor.matmul(
                            ps[:, 0:wd],
                            wq_sb[:, kt, mi * 128 : mi * 128 + 128],
                            xsl(s0, wd)[:, kt, :],
                            start=(kt == 0),
                            stop=(kt == 7),
                        )
                    nc.scalar.activation(
                        qt[:, mi, d0 : d0 + wd],
                        ps[:, 0:wd],
                        AF.Identity,
                        bias=bqk_sb[:, mi : mi + 1],
                    )

            def emit_v(vc):
                for si in range(3):
                    b0 = (vc * 3 + si) * 128
                    for fh in range(2):
                        ps = psAV.tile(
                            [128, 512], F32, tag="a", name=f"vp{vc}_{si}_{fh}"
                        )
                        for kt in range(8):
                            nc.tensor.matmul(
                                ps[:],
                                xsl(b0, 128)[:, kt, :],
                                wv_sb[:, kt, fh * 512 : fh * 512 + 512],
                                start=(kt == 0),
                                stop=(kt == 7),
                            )
                        nc.scalar.activation(
                            vtile[:, vc * 3 + si, fh * 8 : fh * 8 + 8, 0:HD],
                            ps[:].rearrange("p (h d) -> p h d", d=HD),
                            AF.Identity,
                            bias=0.0,
                        )

            # score slot layout per head: pt0 slots = [K0|K8, K1, K2, K3],
            # pt1 slots = [K4, K5, K6, K7]. All q-windows are 256-wide
            # starting at 128K-128 except K0 ([0,128)) and K8 ([896,1024)).
            pts = {}  # h -> (pt0, pt1)

            def emit_scores(h, mid=None):
                mi_h, po = h // 2, (h % 2) * 64
                sc0 = psS.tile([128, 1024], F32, tag="s", name=f"sc0_{h}")
                sc1 = psS.tile([128, 1024], F32, tag="s", name=f"sc1_{h}")
                lhs = lambda K: ktile[po : po + 64, K, mi_h, :]
                rhsq = lambda c0, w: qt[po : po + 64, mi_h, c0 : c0 + w]
                # G0: K0 -> cols 0:128, K8 -> cols 128:256, K1-3 slots 1-3
                nc.tensor.matmul(
                    sc0[:, 0:128], lhs(0), rhsq(0, 128), start=True, stop=True
                )
                nc.tensor.matmul(
                    sc0[:, 128:256], lhs(8), rhsq(896, 128), start=True, stop=True
                )
                for K in (1, 2, 3):
                    nc.tensor.matmul(
                        sc0[:, K * 256 : K * 256 + 256],
                        lhs(K),
                        rhsq(128 * K - 128, 256),
                        start=True,
                        stop=True,
                    )
                if mid is not None:
                    mid()
                for K in (4, 5, 6, 7):
                    j = K - 4
                    nc.tensor.matmul(
                        sc1[:, j * 256 : j * 256 + 256],
                        lhs(K),
                        rhsq(128 * K - 128, 256),
                        start=True,
                        stop=True,
                    )
                pt0 = ptpool.tile([128, 4, 256], BF16, tag="pt0", name=f"pt0_{h}")
                pt1 = ptpool.tile([128, 4, 256], BF16, tag="pt1", name=f"pt1_{h}")
                nc.scalar.activation(
                    pt0[:].rearrange("p g f -> p (g f)"), sc0[:], AF.Exp
                )
                nc.scalar.activation(
                    pt1[:].rearrange("p g f -> p (g f)"), sc1[:], AF.Exp
                )
                nc.gpsimd.tensor_tensor(
                    pt0[:].rearrange("p g f -> p (g f)"),
                    pt0[:].rearrange("p g f -> p (g f)"),
                    m2g0_sb[:],
                    MULT,
                )
                nc.gpsimd.tensor_tensor(
                    pt1[:],
                    pt1[:],
                    m2_sb[:, None, :].to_broadcast((128, 4, 256)),
                    MULT,
                )
                pts[h] = (pt0, pt1)

            def pt_slice(h, K, c0, w):
                pt0, pt1 = pts[h]
                if K == 0:
                    return pt0[:, 0, c0 : c0 + w]
                if K == 8:
                    return pt0[:, 0, 128 + c0 : 128 + c0 + w]
                if K <= 3:
                    return pt0[:, K, c0 : c0 + w]
                return pt1[:, K - 4, c0 : c0 + w]

            def emit_pv(h, psPV):
                mi_h, po = h // 2, (h % 2) * 64
                pvden = psPV.tile([128, 1024], F32, tag="pv", name=f"pv{h}")
                Vh = lambda K: vtile[:, K, h, :]
                for q4 in range(4):
                    q0 = q4 * 256
                    KB, KA, KC = 2 * q4 + 1, 2 * q4, 2 * q4 + 2
                    nc.tensor.matmul(
                        pvden[:, q0 : q0 + 256],
                        Vh(KB),
                        pt_slice(h, KB, 0, 256),
                        start=True,
                        stop=False,
                    )
                    # KA covers [q0, q0+128): offset 128 in its window
                    # (except K0 whose window is [0,128) itself)
                    offA = 0 if KA == 0 else 128
                    nc.tensor.matmul(
                        pvden[:, q0 : q0 + 128],
                        Vh(KA),
                        pt_slice(h, KA, offA, 128),
                        start=False,
                        stop=False,
                    )
                    # KC covers [q0+128, q0+256): offset 0 in its window
                    nc.tensor.matmul(
                        pvden[:, q0 + 128 : q0 + 256],
                        Vh(KC),
                        pt_slice(h, KC, 0, 128),
                        start=False,
                        stop=True,
                    )
                # normalize: at[rows, mi_h, :] = pv * (1/den). den is
                # replicated over psum partitions 64:128 by the ones cols.
                # DVE divide is not an ISA op, so reciprocal + mult
                # (single-PSUM-operand each, mixed partition bases legal).
                rec = denpool.tile([64, 1024], F32, tag="d", name=f"rc{h}")
                nc.vector.reciprocal(rec[:], pvden[64:128, :])
                nc.vector.tensor_tensor(
                    at[po : po + 64, mi_h, :],
                    pvden[0:64, :],
                    rec[:],
                    MULT,
                )

            # pipeline: Q(mi) ahead of head 2*mi, V chunks under heads
            # 0-2, PV trailing GAP heads behind scores
            GAP = 3
            pv_next = [0]

            def _drain(upto):
                while pv_next[0] <= upto:
                    emit_pv(pv_next[0], psPV)
                    pv_next[0] += 1

            emit_q(0)
            emit_scores(0)
            emit_v(0)
            emit_scores(1)
            emit_q(1)
            emit_v(1)
            emit_scores(2)
            emit_v(2)
            for h in range(3, 16):
                if h % 2 == 0:
                    emit_q(h // 2)
                want = h - GAP
                emit_scores(h, mid=(lambda w=want: _drain(w)))
            stageWX.close()

            # ---- stage C: out-projection. Chains for mi 0..3 are split
            # into a (kt 0..5) pass -> `out` and a (kt 6,7) pass -> `out2`
            # (host adds the halves). The kt 0..5 passes only need heads
            # 0..11, so they fill the PE while the DVE normalizes the
            # trailing heads 13..15 instead of idling behind it.
            psC = stageB.enter_context(
                tc.tile_pool(name="psC", bufs=2, space="PSUM")
            )
            ytpool = stageB.enter_context(tc.tile_pool(name="yt", bufs=4))

            def c_steps(ps, mi, sb, kts, start_kt=0, stop_kt=7):
                for kt in kts:
                    nc.tensor.matmul(
                        ps[:],
                        wo_sb[:, kt, mi * 128 : mi * 128 + 128],
                        at[:, kt, sb * 512 : sb * 512 + 512],
                        start=(kt == start_kt),
                        stop=(kt == stop_kt),
                    )

            def c_evac(ps, yt, mi, sb, bias=True):
                nc.scalar.activation(
                    yt[:, sb * 512 : sb * 512 + 512],
                    ps[:],
                    AF.Identity,
                    bias=bo_sb[:, mi : mi + 1] if bias else 0.0,
                )

            def c_dma(yt, mi, dst):
                # one [128, 1024] DMA per mi instead of two halves: fewer
                # dynamically-generated descriptors per call
                r0 = (mi * 128) % dst.shape[0]
                nc.sync.dma_start(dst[r0 : r0 + 128, :], yt[:])

            def c_pass1(mi, sb, yt):
                ps = psC.tile([128, 512], F32, tag="c", name=f"cp{mi}_{sb}a")
                c_steps(ps, mi, sb, range(6), stop_kt=5)
                c_evac(ps, yt, mi, sb)

            yts = {
                mi: ytpool.tile([128, 1024], F32, tag="yt", name=f"yt{mi}")
                for mi in range(4)
            }
            c_pass1(0, 0, yts[0])
            c_pass1(0, 1, yts[0])
            c_dma(yts[0], 0, out)
            _drain(13)
            c_pass1(1, 0, yts[1])
            c_pass1(1, 1, yts[1])
            c_dma(yts[1], 1, out)
            _drain(14)
            c_pass1(2, 0, yts[2])
            c_pass1(2, 1, yts[2])
            c_dma(yts[2], 2, out)
            _drain(15)
            for sb in range(2):
                c_pass1(3, sb, yts[3])
            c_dma(yts[3], 3, out)

            def c_pass2(mi, yt):
                for sb in range(2):
                    ps = psC.tile([128, 512], F32, tag="c", name=f"cp{mi}_{sb}b")
                    c_steps(ps, mi, sb, (6, 7), start_kt=6)
                    c_evac(ps, yt, mi, sb, bias=False)
                c_dma(yt, mi, out2)

            # full chains for mi 4..7 (all heads are normalized by now),
            # interleaved with the (kt 6,7) second passes so the tail
            # evac+DMA of the last passes overlap full-chain PE work.
            for mi in range(4, 8):
                yt = ytpool.tile([128, 1024], F32, tag="yt", name=f"yt{mi}")
                yt2 = ytpool.tile([128, 1024], F32, tag="yt", name=f"yt2_{mi - 4}")
                c_pass2(mi - 4, yt2)
                for sb in range(2):
                    ps = psC.tile([128, 512], F32, tag="c", name=f"cp{mi}_{sb}")
                    c_steps(ps, mi, sb, range(8))
                    c_evac(ps, yt, mi, sb)
                c_dma(yt, mi, out)
            stageB.close()
    nc.compile()
    return nc


def _prep_inputs(x, w_qkv, b_qkv, w_out, b_out):
    x = np.asarray(x, np.float32)
    w_qkv = np.asarray(w_qkv, np.float32)
    b_qkv = np.asarray(b_qkv, np.float32)
    w_out = np.asarray(w_out, np.float32)
    b_out = np.asarray(b_out, np.float32)
    bf16 = _bf16_dtype()

    p_ = np.arange(128)[:, None]
    f_ = np.arange(256)[None, :]
    with np.errstate(over="ignore", under="ignore"):
        m2c = np.where(f_ >= p_, np.exp((p_ - f_).astype(np.float64)), 0.0).astype(bf16)
    scale = np.float32(1.0 / np.sqrt(HD))

    def pack(wT):
        # [D_in, F] -> kt-major [128, 8*F] (one contiguous run/partition)
        a = np.asarray(wT, np.float32).reshape(8, 128, -1)
        return np.ascontiguousarray(
            a.transpose(1, 0, 2).reshape(128, -1)
        ).astype(bf16)

    wq = w_qkv[0:D] * scale
    wqp = pack(wq.T)
    wkT = np.asarray(w_qkv[D : 2 * D].T, np.float32).reshape(8, 128, 1024)
    wkp = np.ascontiguousarray(
        np.concatenate(
            [
                wkT[:, :, 256 * p : 256 * p + 256].transpose(1, 0, 2).reshape(128, -1)
                for p in range(4)
            ],
            axis=1,
        )
    ).astype(bf16)
    wvp = pack(w_qkv[2 * D :].T)
    wop = pack(w_out.T)
    XCH = [(0, 256), (256, 384), (640, 512)]
    bq = b_qkv[0:D] * scale
    bqk_h = np.ascontiguousarray(
        np.concatenate([bq, b_qkv[D : 2 * D]]).reshape(16, 128).T
    )
    # V-bias folds into the out-proj bias: softmax weights sum to 1,
    # so attn(v + bv) @ Wo^T + bo == attn(v) @ Wo^T + (bo + Wo @ bv)
    bv = b_qkv[2 * D :].astype(np.float64)
    bo_eff = (b_out.astype(np.float64) + w_out.astype(np.float64) @ bv).astype(
        np.float32
    )
    bo_h = np.ascontiguousarray(bo_eff.reshape(8, 128).T)

    in_maps = []
    for c in range(NCORES):
        b, qh = c // 2, c % 2
        Q0 = qh * QH
        xw = np.zeros((KW, D), np.float32)
        lo = Q0 - 128
        src_lo = max(lo, 0)
        xw[src_lo - lo : KW] = x[b, src_lo : Q0 + QH]
        m2e = (
            np.asarray(m2c[:, 128:256])
            if qh == 1
            else np.zeros((128, 128), np.float32).astype(bf16)
        )
        # G0 mask: [K0-edge | K8 (= m2c[:, :128]) | m2c | m2c | m2c]
        m2g0 = np.ascontiguousarray(
            np.concatenate([m2e, m2c[:, 0:128], m2c, m2c, m2c], axis=1)
        ).astype(bf16)
        xT8 = np.asarray(xw.T, np.float32).reshape(8, 128, KW)
        xpk = np.ascontiguousarray(
            np.concatenate(
                [
                    xT8[:, :, s0 : s0 + wd].transpose(1, 0, 2).reshape(128, -1)
                    for s0, wd in XCH
                ],
                axis=1,
            )
        ).astype(bf16)
        in_maps.append(
            {
                "xpk": xpk,
                "wqp": wqp,
                "wkp": wkp,
                "wvp": wvp,
                "wop": wop,
                "m2g0": m2g0,
                "m2c": m2c,
                "bqk": bqk_h,
                "bo": bo_h,
            }
        )
    return in_maps


def _get_runner():
    if "runner" in _CACHE:
        return _CACHE["runner"]
    import jax
    from jax.sharding import Mesh, PartitionSpec, NamedSharding
    from jax.experimental.shard_map import shard_map
    import concourse.mybir as mybir
    from concourse.bass2jax import (
        _bass_exec_p,
        install_neuronx_cc_hook,
        partition_id_tensor,
    )

    nc = _build()
    install_neuronx_cc_hook()
    partition_name = nc.partition_id_tensor.name if nc.partition_id_tensor else None
    in_names, out_names, out_avals, zero_outs = [], [], [], []
    for alloc in nc.m.functions[0].allocations:
        if not isinstance(alloc, mybir.MemoryLocationSet):
            continue
        name = alloc.memorylocations[0].name
        if alloc.kind == "ExternalInput":
            if name != partition_name:
                in_names.append(name)
        elif alloc.kind == "ExternalOutput":
            shape = tuple(alloc.tensor_shape)
            dtype = mybir.dt.np(alloc.dtype)
            out_names.append(name)
            out_avals.append(jax.core.ShapedArray(shape, dtype))
            zero_outs.append(np.zeros(shape, dtype))
    all_in = list(in_names) + list(out_names)
    if partition_name is not None:
        all_in.append(partition_name)

    def _body(*args):
        operands = list(args)
        if partition_name is not None:
            operands.append(partition_id_tensor())
        outs = _bass_exec_p.bind(
            *operands,
            out_avals=tuple(out_avals),
            in_names=tuple(all_in),
            out_names=tuple(out_names),
            lowering_input_output_aliases=(),
            sim_require_finite=True,
            sim_require_nnan=True,
            nc=nc,
        )
        return tuple(outs)

    devices = jax.devices()[:NCORES]
    mesh = Mesh(np.asarray(devices), ("core",))
    nio = len(in_names) + len(out_names)
    fn = jax.jit(
        shard_map(
            _body,
            mesh=mesh,
            in_specs=(PartitionSpec("core"),) * nio,
            out_specs=(PartitionSpec("core"),) * len(out_names),
            check_rep=False,
        ),
        keep_unused=True,
    )
    runner = {
        "fn": fn,
        "in_names": in_names,
        "out_names": out_names,
        "out_avals": out_avals,
        "zero_outs": zero_outs,
        "sharding": NamedSharding(mesh, PartitionSpec("core")),
    }
    _CACHE["runner"] = runner
    return runner


def kernel(x, w_qkv, b_qkv, w_out, b_out):
    import jax

    in_maps = _prep_inputs(x, w_qkv, b_qkv, w_out, b_out)
    r = _get_runner()
    n = NCORES
    concat_in = [
        np.concatenate([np.asarray(in_maps[c][name]) for c in range(n)], axis=0)
        for name in r["in_names"]
    ]
    concat_zero = [
        np.zeros((n * z.shape[0], *z.shape[1:]), z.dtype) for z in r["zero_outs"]
    ]
    args = [jax.device_put(a, r["sharding"]) for a in concat_in + concat_zero]
    outs = r["fn"](*args)
    jax.block_until_ready(outs)
    oname = r["out_names"].index("out")
    o2name = r["out_names"].index("out2")
    full = np.asarray(outs[oname]).reshape(n, D, QH).copy()
    full2 = np.asarray(outs[o2name]).reshape(n, D // 2, QH)
    full[:, : D // 2] += full2  # kt 6,7 partial for mi 0..3
    y = np.empty((B, S, D), np.float32)
    for b in range(B):
        yt = np.concatenate([full[2 * b], full[2 * b + 1]], axis=1)  # [1024, 2048]
        y[b] = yt.T
    return y



# revision 34
# speedup vs baseline: 1.1608x; 1.1608x over previous
"""ALiBi causal attention on 8 TRN2 NeuronCores — no-communication variant.

Sharding: batch (4) x query-half (2) = 8 cores, zero collectives.
Each core receives a HOST-WINDOWED x covering key positions
[Q0-128, Q0+1024) of its batch (front-padded with zeros on even cores).
It computes K/V for the 9-k-tile window, Q for its half (reusing the
same x window — queries are window cols 128:1152), banded causal
attention (ALiBi decay zeroes k < q-127 in bf16), and the out-projection.

Key structure:
- K0 and K8 share one 256-col score slot -> 2 exp ops/head, no G2.
- attn output written straight into SBUF (at tile) by the DVE
  reciprocal+mult normalize; denominator rides the PV matmul via ones
  columns in vtile.
- all big inputs host-packed kt-major so each DMA is one contiguous
  run per partition (128 descriptors, matters under io-level dynamic
  DMA generation), spread over the sync/act/pool queues so the first
  K chain starts ~3.5us in.
- out-proj chains for mi 0-3 split into (kt 0-5) -> out and (kt 6,7)
  -> out2 partial passes (host adds them) so the PE stays busy while
  the DVE normalizes the trailing heads.
- masks on Pool, V/K/Q/C evacs + exp on Act, normalize on DVE.
"""
import numpy as np


def _bf16_dtype():
    import ml_dtypes

    return np.dtype(ml_dtypes.bfloat16)


B, S, D = 4, 2048, 1024
H, HD = 16, 64
NCORES = 8
QH = S // 2          # 1024 queries per core
KW = QH + 128        # 1152 key-window positions per core (9 k-tiles)
NKT = KW // 128      # 9 local k-tiles

_CACHE = {}


def _build():
    import concourse.mybir as mybir
    import concourse.tile as tile
    from concourse import bacc
    from contextlib import ExitStack

    F32 = mybir.dt.float32
    BF16 = mybir.dt.bfloat16
    AF = mybir.ActivationFunctionType
    MULT = mybir.AluOpType.mult
    DIV = mybir.AluOpType.divide

    nc = bacc.Bacc("TRN2", target_bir_lowering=False, debug=False, num_devices=NCORES)

    # All big inputs are host-packed [128, kt-major] so every DMA is one
    # contiguous run per partition: 128 descriptors instead of 1024. The
    # compile uses io-level dynamic DMA generation, so descriptors are
    # rebuilt every call - descriptor count is per-call overhead.
    xpk = nc.dram_tensor("xpk", [128, 8 * KW], BF16, kind="ExternalInput").ap()
    wqp = nc.dram_tensor("wqp", [128, 8 * D], BF16, kind="ExternalInput").ap()
    wkp = nc.dram_tensor("wkp", [128, 8 * D], BF16, kind="ExternalInput").ap()
    wvp = nc.dram_tensor("wvp", [128, 8 * D], BF16, kind="ExternalInput").ap()
    wop = nc.dram_tensor("wop", [128, 8 * D], BF16, kind="ExternalInput").ap()
    m2g0 = nc.dram_tensor("m2g0", [128, 1024], BF16, kind="ExternalInput").ap()
    m2c = nc.dram_tensor("m2c", [128, 256], BF16, kind="ExternalInput").ap()
    bqk = nc.dram_tensor("bqk", [128, 16], F32, kind="ExternalInput").ap()
    bo = nc.dram_tensor("bo", [128, 8], F32, kind="ExternalInput").ap()
    out = nc.dram_tensor("out", [D, QH], F32, kind="ExternalOutput").ap()
    # partial out-proj contributions (kt 6,7) for mi 0..3; host adds them.
    out2 = nc.dram_tensor("out2", [D // 2, QH], F32, kind="ExternalOutput").ap()

    # x is packed as three kt-major chunks (cols 0:256, 256:640, 640:1152)
    # so each chunk loads contiguously; chunk boundaries are chosen so
    # K, Q and V matmul column ranges never straddle a chunk.
    XCH = [(0, 256), (256, 384), (640, 512)]
    xsrc = []
    off = 0
    for s0, wd in XCH:
        xsrc.append(
            xpk[:, off : off + 8 * wd].rearrange("p (kt f) -> p kt f", kt=8)
        )
        off += 8 * wd
    wq3 = wqp.rearrange("p (kt f) -> p kt f", kt=8)     # [128, 8, 1024]
    wv3 = wvp.rearrange("p (kt f) -> p kt f", kt=8)
    wo3 = wop.rearrange("p (kt f) -> p kt f", kt=8)
    # wk is packed as four mi-pair chunks of [8 kt x 256]
    wksrc = [
        wkp[:, 2048 * p : 2048 * p + 2048].rearrange("p2 (kt f) -> p2 kt f", kt=8)
        for p in range(4)
    ]

    with tile.TileContext(nc) as tc:
        with (
            tc.tile_pool(name="const", bufs=1) as cpool,
            tc.tile_pool(name="big", bufs=1) as big,
            ExitStack() as outer,
        ):
            m2g0_sb = cpool.tile([128, 1024], BF16)
            m2_sb = cpool.tile([128, 256], BF16)
            bqk_sb = cpool.tile([128, 16], F32)
            bo_sb = cpool.tile([128, 8], F32)

            # persistent tiles
            ktile = big.tile([128, NKT, 8, 128], BF16, tag="kt", name="ktile")
            vtile = big.tile([128, NKT, H, 2 * HD], BF16, tag="vt", name="vtile")
            qt = big.tile([128, 8, QH], BF16, tag="qt", name="qtile")
            at = big.tile([128, 8, QH], BF16, tag="at", name="atile")
            wo_sb = big.tile([128, 8, D], BF16, tag="wo", name="wo_sb")

            # (vtile ones memsets are emitted later, after the startup DMAs,
            # so they don't block the DVE/Pool queues at t=0)

            # LIFO pool staging: psS/pt (whole stage B) below, then
            # psAV/w/x (until V-proj done), then psPV, then psC/yt.
            stageB = outer.enter_context(ExitStack())
            psS = stageB.enter_context(
                tc.tile_pool(name="psS", bufs=2, space="PSUM")
            )
            ptpool = stageB.enter_context(tc.tile_pool(name="pt", bufs=4))
            denpool = stageB.enter_context(tc.tile_pool(name="den", bufs=3))
            psPV = stageB.enter_context(
                tc.tile_pool(name="psPV", bufs=1, space="PSUM")
            )
            stageWX = outer.enter_context(ExitStack())
            psAV = stageWX.enter_context(
                tc.tile_pool(name="psAV", bufs=2, space="PSUM")
            )
            wpool = stageWX.enter_context(tc.tile_pool(name="w", bufs=1))
            xpool = stageWX.enter_context(tc.tile_pool(name="xin", bufs=1))

            wq_sb = wpool.tile([128, 8, D], BF16, tag="wq", name="wq_sb")
            wv_sb = wpool.tile([128, 8, D], BF16, tag="wv", name="wv_sb")
            wk_t = [
                wpool.tile([128, 8, 256], BF16, tag=f"wk{p}", name=f"wk{p}")
                for p in range(4)
            ]
            xc = [
                xpool.tile([128, 8, wd], BF16, tag=f"xc{i}", name=f"xc{i}")
                for i, (s0, wd) in enumerate(XCH)
            ]

            def xsl(c, w):
                """x slice [128, 8, w] at global window col c."""
                for (s0, wd), t in zip(XCH, xc):
                    if s0 <= c and c + w <= s0 + wd:
                        return t[:, :, c - s0 : c - s0 + w]
                raise AssertionError((c, w))

            # input DMAs, spread over the three DMA-capable queues so the
            # first K chain (x cols 0:256 + wk pair 0) is ready ~3.5us in
            # and later chains' operands stream in just ahead of use:
            #   pool: wk pair chunks, wq, wo, then a vtile ones memset
            #   sync: x chunks, bqk, wv + consts
            #   act:  only the exp-table load, pulled to t=0 by a 1-elem
            #         warm activation, so it doesn't delay the K evacs
            warm = cpool.tile([1, 1], F32)
            nc.vector.memset(warm[:], 0.0)
            nc.scalar.activation(warm[:], warm[:], AF.Exp)
            for p in range(4):
                nc.gpsimd.dma_start(wk_t[p][:], wksrc[p])
            nc.sync.dma_start(xc[0][:], xsrc[0])
            nc.sync.dma_start(bqk_sb[:], bqk)
            nc.sync.dma_start(xc[1][:], xsrc[1])
            nc.sync.dma_start(xc[2][:], xsrc[2])
            nc.gpsimd.dma_start(wq_sb[:], wq3)
            nc.sync.dma_start(wv_sb[:], wv3)
            nc.sync.dma_start(m2g0_sb[:], m2g0)
            nc.sync.dma_start(m2_sb[:], m2c)
            nc.sync.dma_start(bo_sb[:], bo)
            nc.gpsimd.dma_start(wo_sb[:], wo3)
            # ones columns for the replicated-denominator PV trick
            nc.vector.memset(vtile[:, :, 0:8, HD : 2 * HD], 1.0)
            nc.gpsimd.memset(vtile[:, :, 8:16, HD : 2 * HD], 1.0)

            # ---- K-projection: key chunks (= x chunks) x 8 m-tiles. The
            # first chunk is small so the first chain starts as soon as
            # the lean startup DMA prefix lands. ----
            for ci, (s0, kw) in enumerate(XCH):
                nt = kw // 128
                for mi in range(8):
                    ps = psAV.tile([128, 512], F32, tag="a", name=f"kp{ci}_{mi}")
                    for kt in range(8):
                        nc.tensor.matmul(
                            ps[:, 0:kw],
                            wk_t[mi // 2][:, kt, (mi % 2) * 128 : (mi % 2) * 128 + 128],
                            xsl(s0, kw)[:, kt, :],
                            start=(kt == 0),
                            stop=(kt == 7),
                        )
                    nc.scalar.activation(
                        ktile[:, s0 // 128 : s0 // 128 + nt, mi, :],
                        ps[:, 0:kw].rearrange("p (t f) -> p t f", t=nt),
                        AF.Identity,
                        bias=bqk_sb[:, 8 + mi : 8 + mi + 1],
                    )
            # ---- Q-projection (deferred): queries are window cols
            # 128:1152, split per x chunk; Q(mi) is emitted just before
            # head 2*mi so the PE has surplus work against the Act-exp
            # critical path ----
            QCH = [(128, 128, 0), (256, 384, 128), (640, 512, 512)]

            def emit_q(mi):
                for qc, (s0, wd, d0) in enumerate(QCH):
                    ps = psAV.tile([128, 512], F32, tag="a", name=f"qp{qc}_{mi}")
                    for kt in range(8):
                        nc.tensor.matmul(
                            ps[:, 0:wd],
                            wq_sb[:, kt, mi * 128 : mi * 128 + 128],
                            xsl(s0, wd)[:, kt, :],
                            start=(kt == 0),
                            stop=(kt == 7),
                        )
                    nc.scalar.activation(
                        qt[:, mi, d0 : d0 + wd],
                        ps[:, 0:wd],
                        AF.Identity,
                        bias=bqk_sb[:, mi : mi + 1],
                    )

            def emit_v(vc):
                for si in range(3):
                    b0 = (vc * 3 + si) * 128
                    for fh in range(2):
                        ps = psAV.tile(
                            [128, 512], F32, tag="a", name=f"vp{vc}_{si}_{fh}"
                        )
                        for kt in range(8):
                            nc.tensor.matmul(
                                ps[:],
                                xsl(b0, 128)[:, kt, :],
                                wv_sb[:, kt, fh * 512 : fh * 512 + 512],
                                start=(kt == 0),
                                stop=(kt == 7),
                            )
                        nc.scalar.activation(
                            vtile[:, vc * 3 + si, fh * 8 : fh * 8 + 8, 0:HD],
                            ps[:].rearrange("p (h d) -> p h d", d=HD),
                            AF.Identity,
                            bias=0.0,
                        )

            # score slot layout per head: pt0 slots = [K0|K8, K1, K2, K3],
            # pt1 slots = [K4, K5, K6, K7]. All q-windows are 256-wide
            # starting at 128K-128 except K0 ([0,128)) and K8 ([896,1024)).
            pts = {}  # h -> (pt0, pt1)

            def emit_scores(h, mid=None):
                mi_h, po = h // 2, (h % 2) * 64
                sc0 = psS.tile([128, 1024], F32, tag="s", name=f"sc0_{h}")
                sc1 = psS.tile([128, 1024], F32, tag="s", name=f"sc1_{h}")
                lhs = lambda K: ktile[po : po + 64, K, mi_h, :]
                rhsq = lambda c0, w: qt[po : po + 64, mi_h, c0 : c0 + w]
                # G0: K0 -> cols 0:128, K8 -> cols 128:256, K1-3 slots 1-3
                nc.tensor.matmul(
                    sc0[:, 0:128], lhs(0), rhsq(0, 128), start=True, stop=True
                )
                nc.tensor.matmul(
                    sc0[:, 128:256], lhs(8), rhsq(896, 128), start=True, stop=True
                )
                for K in (1, 2, 3):
                    nc.tensor.matmul(
                        sc0[:, K * 256 : K * 256 + 256],
                        lhs(K),
                        rhsq(128 * K - 128, 256),
                        start=True,
                        stop=True,
                    )
                if mid is not None:
                    mid()
                for K in (4, 5, 6, 7):
                    j = K - 4
                    nc.tensor.matmul(
                        sc1[:, j * 256 : j * 256 + 256],
                        lhs(K),
                        rhsq(128 * K - 128, 256),
                        start=True,
                        stop=True,
                    )
                pt0 = ptpool.tile([128, 4, 256], BF16, tag="pt0", name=f"pt0_{h}")
                pt1 = ptpool.tile([128, 4, 256], BF16, tag="pt1", name=f"pt1_{h}")
                nc.scalar.activation(
                    pt0[:].rearrange("p g f -> p (g f)"), sc0[:], AF.Exp
                )
                nc.scalar.activation(
                    pt1[:].rearrange("p g f -> p (g f)"), sc1[:], AF.Exp
                )
                nc.gpsimd.tensor_tensor(
                    pt0[:].rearrange("p g f -> p (g f)"),
                    pt0[:].rearrange("p g f -> p (g f)"),
                    m2g0_sb[:],
                    MULT,
                )
                nc.gpsimd.tensor_tensor(
                    pt1[:],
                    pt1[:],
                    m2_sb[:, None, :].to_broadcast((128, 4, 256)),
                    MULT,
                )
                pts[h] = (pt0, pt1)

            def pt_slice(h, K, c0, w):
                pt0, pt1 = pts[h]
                if K == 0:
                    return pt0[:, 0, c0 : c0 + w]
                if K == 8:
                    return pt0[:, 0, 128 + c0 : 128 + c0 + w]
                if K <= 3:
                    return pt0[:, K, c0 : c0 + w]
                return pt1[:, K - 4, c0 : c0 + w]

            def emit_pv(h, psPV):
                mi_h, po = h // 2, (h % 2) * 64
                pvden = psPV.tile([128, 1024], F32, tag="pv", name=f"pv{h}")
                Vh = lambda K: vtile[:, K, h, :]
                for q4 in range(4):
                    q0 = q4 * 256
                    KB, KA, KC = 2 * q4 + 1, 2 * q4, 2 * q4 + 2
                    nc.tensor.matmul(
                        pvden[:, q0 : q0 + 256],
                        Vh(KB),
                        pt_slice(h, KB, 0, 256),
                        start=True,
                        stop=False,
                    )
                    # KA covers [q0, q0+128): offset 128 in its window
                    # (except K0 whose window is [0,128) itself)
                    offA = 0 if KA == 0 else 128
                    nc.tensor.matmul(
                        pvden[:, q0 : q0 + 128],
                        Vh(KA),
                        pt_slice(h, KA, offA, 128),
                        start=False,
                        stop=False,
                    )
                    # KC covers [q0+128, q0+256): offset 0 in its window
                    nc.tensor.matmul(
                        pvden[:, q0 + 128 : q0 + 256],
                        Vh(KC),
                        pt_slice(h, KC, 0, 128),
                        start=False,
                        stop=True,
                    )
                # normalize: at[rows, mi_h, :] = pv * (1/den). den is
                # replicated over psum partitions 64:128 by the ones cols.
                # DVE divide is not an ISA op, so reciprocal + mult
                # (single-PSUM-operand each, mixed partition bases legal).
                rec = denpool.tile([64, 1024], F32, tag="d", name=f"rc{h}")
                nc.vector.reciprocal(rec[:], pvden[64:128, :])
                nc.vector.tensor_tensor(
                    at[po : po + 64, mi_h, :],
                    pvden[0:64, :],
                    rec[:],
                    MULT,
                )

            # pipeline: Q(mi) ahead of head 2*mi, V chunks under heads
            # 0-2, PV trailing GAP heads behind scores
            GAP = 3
            pv_next = [0]

            def _drain(upto):
                while pv_next[0] <= upto:
                    emit_pv(pv_next[0], psPV)
                    pv_next[0] += 1

            emit_q(0)
            emit_scores(0)
            emit_v(0)
            emit_scores(1)
            emit_q(1)
            emit_v(1)
            emit_scores(2)
            emit_v(2)
            for h in range(3, 16):
                if h % 2 == 0:
                    emit_q(h // 2)
                want = h - GAP
                emit_scores(h, mid=(lambda w=want: _drain(w)))
            stageWX.close()

            # ---- stage C: out-projection. Chains for mi 0..3 are split
            # into a (kt 0..5) pass -> `out` and a (kt 6,7) pass -> `out2`
            # (host adds the halves). The kt 0..5 passes only need heads
            # 0..11, so they fill the PE while the DVE normalizes the
            # trailing heads 13..15 instead of idling behind it.
            psC = stageB.enter_context(
                tc.tile_pool(name="psC", bufs=2, space="PSUM")
            )
            ytpool = stageB.enter_context(tc.tile_pool(name="yt", bufs=4))

            def c_steps(ps, mi, sb, kts, start_kt=0, stop_kt=7):
                for kt in kts:
                    nc.tensor.matmul(
                        ps[:],
                        wo_sb[:, kt, mi * 128 : mi * 128 + 128],
                        at[:, kt, sb * 512 : sb * 512 + 512],
                        start=(kt == start_kt),
                        stop=(kt == stop_kt),
                    )

            def c_finish(ps, yt, mi, sb, dst=None, bias=True):
                nc.scalar.activation(
                    yt[:, sb * 512 : sb * 512 + 512],
                    ps[:],
                    AF.Identity,
                    bias=bo_sb[:, mi : mi + 1] if bias else 0.0,
                )
                d = out if dst is None else dst
                r0 = (mi * 128) % d.shape[0]
                nc.sync.dma_start(
                    d[r0 : r0 + 128, sb * 512 : sb * 512 + 512],
                    yt[:, sb * 512 : sb * 512 + 512],
                )

            def c_pass1(mi, sb, yt):
                ps = psC.tile([128, 512], F32, tag="c", name=f"cp{mi}_{sb}a")
                c_steps(ps, mi, sb, range(6), stop_kt=5)
                c_finish(ps, yt, mi, sb)

            yts = {
                mi: ytpool.tile([128, 1024], F32, tag="yt", name=f"yt{mi}")
                for mi in range(4)
            }
            c_pass1(0, 0, yts[0])
            c_pass1(0, 1, yts[0])
            _drain(13)
            c_pass1(1, 0, yts[1])
            c_pass1(1, 1, yts[1])
            _drain(14)
            c_pass1(2, 0, yts[2])
            c_pass1(2, 1, yts[2])
            _drain(15)
            for sb in range(2):
                c_pass1(3, sb, yts[3])

            def c_pass2(mi, yt):
                for sb in range(2):
                    ps = psC.tile([128, 512], F32, tag="c", name=f"cp{mi}_{sb}b")
                    c_steps(ps, mi, sb, (6, 7), start_kt=6)
                    c_finish(ps, yt, mi, sb, dst=out2, bias=False)

            # full chains for mi 4..7 (all heads are normalized by now),
            # interleaved with the (kt 6,7) second passes so the tail
            # evac+DMA of the last passes overlap full-chain PE work.
            for mi in range(4, 8):
                yt = ytpool.tile([128, 1024], F32, tag="yt", name=f"yt{mi}")
                yt2 = ytpool.tile([128, 1024], F32, tag="yt", name=f"yt2_{mi - 4}")
                c_pass2(mi - 4, yt2)
                for sb in range(2):
                    ps = psC.tile([128, 512], F32, tag="c", name=f"cp{mi}_{sb}")
                    c_steps(ps, mi, sb, range(8))
                    c_finish(ps, yt, mi, sb)
            stageB.close()
    nc.compile()
    return nc


def _prep_inputs(x, w_qkv, b_qkv, w_out, b_out):
    x = np.asarray(x, np.float32)
    w_qkv = np.asarray(w_qkv, np.float32)
    b_qkv = np.asarray(b_qkv, np.float32)
    w_out = np.asarray(w_out, np.float32)
    b_out = np.asarray(b_out, np.float32)
    bf16 = _bf16_dtype()

    p_ = np.arange(128)[:, None]
    f_ = np.arange(256)[None, :]
    with np.errstate(over="ignore", under="ignore"):
        m2c = np.where(f_ >= p_, np.exp((p_ - f_).astype(np.float64)), 0.0).astype(bf16)
    scale = np.float32(1.0 / np.sqrt(HD))

    def pack(wT):
        # [D_in, F] -> kt-major [128, 8*F] (one contiguous run/partition)
        a = np.asarray(wT, np.float32).reshape(8, 128, -1)
        return np.ascontiguousarray(
            a.transpose(1, 0, 2).reshape(128, -1)
        ).astype(bf16)

    wq = w_qkv[0:D] * scale
    wqp = pack(wq.T)
    wkT = np.asarray(w_qkv[D : 2 * D].T, np.float32).reshape(8, 128, 1024)
    wkp = np.ascontiguousarray(
        np.concatenate(
            [
                wkT[:, :, 256 * p : 256 * p + 256].transpose(1, 0, 2).reshape(128, -1)
                for p in range(4)
            ],
            axis=1,
        )
    ).astype(bf16)
    wvp = pack(w_qkv[2 * D :].T)
    wop = pack(w_out.T)
    XCH = [(0, 256), (256, 384), (640, 512)]
    bq = b_qkv[0:D] * scale
    bqk_h = np.ascontiguousarray(
        np.concatenate([bq, b_qkv[D : 2 * D]]).reshape(16, 128).T
    )
    # V-bias folds into the out-proj bias: softmax weights sum to 1,
    # so attn(v + bv) @ Wo^T + bo == attn(v) @ Wo^T + (bo + Wo @ bv)
    bv = b_qkv[2 * D :].astype(np.float64)
    bo_eff = (b_out.astype(np.float64) + w_out.astype(np.float64) @ bv).astype(
        np.float32
    )
    bo_h = np.ascontiguousarray(bo_eff.reshape(8, 128).T)

    in_maps = []
    for c in range(NCORES):
        b, qh = c // 2, c % 2
        Q0 = qh * QH
        xw = np.zeros((KW, D), np.float32)
        lo = Q0 - 128
        src_lo = max(lo, 0)
        xw[src_lo - lo : KW] = x[b, src_lo : Q0 + QH]
        m2e = (
            np.asarray(m2c[:, 128:256])
            if qh == 1
            else np.zeros((128, 128), np.float32).astype(bf16)
        )
        # G0 mask: [K0-edge | K8 (= m2c[:, :128]) | m2c | m2c | m2c]
        m2g0 = np.ascontiguousarray(
            np.concatenate([m2e, m2c[:, 0:128], m2c, m2c, m2c], axis=1)
        ).astype(bf16)
        xT8 = np.asarray(xw.T, np.float32).reshape(8, 128, KW)
        xpk = np.ascontiguousarray(
            np.concatenate(
                [
                    xT8[:, :, s0 : s0 + wd].transpose(1, 0, 2).reshape(128, -1)
                    for s0, wd in XCH
                ],
                axis=1,
            )
        ).astype(bf16)
        in_maps.append(
            {
                "xpk": xpk,
                "wqp": wqp,
                "wkp": wkp,
                "wvp": wvp,
                "wop": wop,
                "m2g0": m2g0,
                "m2c": m2c,
                "bqk": bqk_h,
                "bo": bo_h,
            }
        )
    return in_maps


def _get_runner():
    if "runner" in _CACHE:
        return _CACHE["runner"]
    import jax
    from jax.sharding import Mesh, PartitionSpec, NamedSharding
    from jax.experimental.shard_map import shard_map
    import concourse.mybir as mybir
    from concourse.bass2jax import (
        _bass_exec_p,
        install_neuronx_cc_hook,
        partition_id_tensor,
    )

    nc = _build()
    install_neuronx_cc_hook()
    partition_name = nc.partition_id_tensor.name if nc.partition_id_tensor else None
    in_names, out_names, out_avals, zero_outs = [], [], [], []
    for alloc in nc.m.functions[0].allocations:
        if not isinstance(alloc, mybir.MemoryLocationSet):
            continue
        name = alloc.memorylocations[0].name
        if alloc.kind == "ExternalInput":
            if name != partition_name:
                in_names.append(name)
        elif alloc.kind == "ExternalOutput":
            shape = tuple(alloc.tensor_shape)
            dtype = mybir.dt.np(alloc.dtype)
            out_names.append(name)
            out_avals.append(jax.core.ShapedArray(shape, dtype))
            zero_outs.append(np.zeros(shape, dtype))
    all_in = list(in_names) + list(out_names)
    if partition_name is not None:
        all_in.append(partition_name)

    def _body(*args):
        operands = list(args)
        if partition_name is not None:
            operands.append(partition_id_tensor())
        outs = _bass_exec_p.bind(
            *operands,
            out_avals=tuple(out_avals),
            in_names=tuple(all_in),
            out_names=tuple(out_names),
            lowering_input_output_aliases=(),
            sim_require_finite=True,
            sim_require_nnan=True,
            nc=nc,
        )
        return tuple(outs)

    devices = jax.devices()[:NCORES]
    mesh = Mesh(np.asarray(devices), ("core",))
    nio = len(in_names) + len(out_names)
    fn = jax.jit(
        shard_map(
            _body,
            mesh=mesh,
            in_specs=(PartitionSpec("core"),) * nio,
            out_specs=(PartitionSpec("core"),) * len(out_names),
            check_rep=False,
        ),
        keep_unused=True,
    )
    runner = {
        "fn": fn,
        "in_names": in_names,
        "out_names": out_names,
        "out_avals": out_avals,
        "zero_outs": zero_outs,
        "sharding": NamedSharding(mesh, PartitionSpec("core")),
    }
    _CACHE["runner"] = runner
    return runner


def kernel(x, w_qkv, b_qkv, w_out, b_out):
    import jax

    in_maps = _prep_inputs(x, w_qkv, b_qkv, w_out, b_out)
    r = _get_runner()
    n = NCORES
    concat_in = [
        np.concatenate([np.asarray(in_maps[c][name]) for c in range(n)], axis=0)
        for name in r["in_names"]
    ]
    concat_zero = [
        np.zeros((n * z.shape[0], *z.shape[1:]), z.dtype) for z in r["zero_outs"]
    ]
    args = [jax.device_put(a, r["sharding"]) for a in concat_in + concat_zero]
    outs = r["fn"](*args)
    jax.block_until_ready(outs)
    oname = r["out_names"].index("out")
    o2name = r["out_names"].index("out2")
    full = np.asarray(outs[oname]).reshape(n, D, QH).copy()
    full2 = np.asarray(outs[o2name]).reshape(n, D // 2, QH)
    full[:, : D // 2] += full2  # kt 6,7 partial for mi 0..3
    y = np.empty((B, S, D), np.float32)
    for b in range(B):
        yt = np.concatenate([full[2 * b], full[2 * b + 1]], axis=1)  # [1024, 2048]
        y[b] = yt.T
    return y

